# revision 19
# baseline (speedup 1.0000x reference)
"""Bilinear edge decoder on 8 TRN2 NeuronCores.

out[e] = sigmoid( z[ei[1,e]] @ W @ z[ei[0,e]] )  for e in [0, 600000)

Strategy (edge-sharded data parallel, dma_gather):
  - Split the 600k edges evenly across 8 cores (75k each); replicate z, W.
  - The HW gather primitive (InstDMAGatherAnt, GPSIMD 'mlp' library) uses
    int16 row indices, so z is treated as 4 sub-tables of 25000 rows.
    Each core's edges are classified into 16 classes by the pair
    (j_subtable, i_subtable) so that, within a class, both endpoint
    gathers use a single sub-table each and share one slot order.
  - Classes are padded (with row-0 dummy edges) to shared static
    capacities (max over cores, rounded to 512) so all 8 cores run one
    SPMD program; the capacities come from the actual input, and the
    program is compiled per capacity signature inside kernel().
  - Per chunk (<=4096 edges) of a class: two dma_gathers fetch z_j and
    z_i rows into [128, slots, 128] tiles (edge q -> partition q%128,
    slot q//128). Per 128-edge slot: PE-transpose z_j, matmul with W,
    DVE multiply with z_i and reduce over features, ACT sigmoid, DMA out.
  - Walrus codegen allows a single sync wait per TPB instruction;
    _split_multi_waits legalizes the Tile-scheduled program by splitting
    extra waits into standalone InstEventSemaphore ops.
"""

import numpy as np

N_NODES = 100000
D = 128
E = 600000
NCORES = 8
EPC = E // NCORES           # 75000 edges per core
NSUB = 4
SUBROWS = 25000             # fits int16 index range
NCLS = NSUB * NSUB
CHUNK = 1024                # max edges per gather op (Q7 gather limit ~1024 idx)
DMA_SCRATCH = 16384         # SWDGE descriptor ring bytes (1024 descs per 16KB)
PADQ = 512                  # class capacity quantum (keeps 4-slot groups even)
GRP = 4                     # slots batched per PSUM bank (512 f32)

_CACHE = {}


def _split_multi_waits(nc):
    """Walrus codegen allows at most one sync wait per TPB instruction.
    Split any instruction with multiple sem-ge waits into preceding
    single-wait InstEventSemaphore ops on the same engine."""
    import concourse.mybir as mybir

    n = 0
    for f in nc.m.functions:
        for blk in f.blocks:
            new = []
            for inst in blk.instructions:
                si = inst.sync_info
                if (
                    si is not None
                    and si.on_wait
                    and len(si.on_wait) > 1
                    and all(
                        w.wait_mode == "sem-ge-imm" and w.wait_reg is None
                        for w in si.on_wait
                    )
                ):
                    waits = list(si.on_wait)
                    for w in waits[:-1]:
                        ev = mybir.InstEventSemaphore(
                            name=f"EVSPLIT-{n}", ins=[], outs=[]
                        )
                        n += 1
                        ev.engine = inst.engine
                        ev.sync_info = mybir.SyncInfo(on_wait=[w], on_update=[])
                        new.append(ev)
                    inst.sync_info = mybir.SyncInfo(
                        on_wait=[waits[-1]], on_update=list(si.on_update)
                    )
                new.append(inst)
            blk.instructions = new
    return n


def _chunks_of(cap):
    out = []
    left = cap
    while left > 0:
        s = min(CHUNK, left)
        out.append(s)
        left -= s
    return out


def _build_program(caps):
    import concourse.bass as bass
    import concourse.mybir as mybir
    import concourse.tile as tile
    from concourse import library_config

    f32 = mybir.dt.float32
    i16 = mybir.dt.int16

    tot = sum(caps)
    tot_slots = tot // 128
    idx_cols = tot // 16

    nc = bass.Bass("TRN2", target_bir_lowering=False, debug=False,
                   num_devices=NCORES,
                   dynamic_dma_scratch_size=DMA_SCRATCH)

    z_d = nc.dram_tensor("z", [N_NODES, D], f32, kind="ExternalInput")
    w_d = nc.dram_tensor("w", [D, D], f32, kind="ExternalInput")
    id_d = nc.dram_tensor("ident", [D, D], f32, kind="ExternalInput")
    jx_d = nc.dram_tensor("jx", [128, idx_cols], i16, kind="ExternalInput")
    ix_d = nc.dram_tensor("ix", [128, idx_cols], i16, kind="ExternalInput")
    out_d = nc.dram_tensor("out", [128, tot_slots], f32, kind="ExternalOutput")

    with tile.TileContext(nc) as tc:
        with (
            tc.tile_pool(name="const", bufs=1) as constp,
            tc.tile_pool(name="g", bufs=2) as gp,
            tc.tile_pool(name="zjt", bufs=3) as zjtp,
            tc.tile_pool(name="mms", bufs=3) as mmsp,
            tc.tile_pool(name="prod", bufs=3) as prodp,
            tc.tile_pool(name="acc", bufs=2) as accp,
            tc.tile_pool(name="scr", bufs=1, space="PSUM") as scrp,
            tc.tile_pool(name="pst", bufs=3, space="PSUM") as pst,
            tc.tile_pool(name="psm", bufs=3, space="PSUM") as psm,
        ):
            nc.gpsimd.load_library(library_config.mlp)

            ident = constp.tile([128, 128], f32)
            nc.sync.dma_start(ident[:], id_d[:, :])
            w_sb = constp.tile([128, 128], f32)
            nc.sync.dma_start(w_sb[:], w_d[:, :])
            jx_sb = constp.tile([128, idx_cols], i16)
            nc.sync.dma_start(jx_sb[:], jx_d[:, :])
            ix_sb = constp.tile([128, idx_cols], i16)
            nc.sync.dma_start(ix_sb[:], ix_d[:, :])

            # dummy PE ops: absorb the constant-load DMA waits once
            scr = scrp.tile([128, 128], f32)
            nc.tensor.transpose(scr[:], ident[:], ident[:])
            scr2 = scrp.tile([128, 128], f32, tag="scr2")
            nc.tensor.matmul(scr2[:], lhsT=w_sb[:], rhs=w_sb[:],
                             start=True, stop=True)

            # one shared register per distinct chunk size (Pool registers
            # are scarce; to_reg per gather would exhaust them)
            size_regs = {}
            for cls in range(NCLS):
                for S in _chunks_of(caps[cls]):
                    if S not in size_regs:
                        size_regs[S] = nc.gpsimd.to_reg(S)

            base = 0
            for cls in range(NCLS):
                bj, bi = divmod(cls, NSUB)
                zj_tab = z_d[bj * SUBROWS:(bj + 1) * SUBROWS, :]
                zi_tab = z_d[bi * SUBROWS:(bi + 1) * SUBROWS, :]
                for S in _chunks_of(caps[cls]):
                    slots = S // 128
                    cb = base // 16
                    gj = gp.tile([128, S], f32, tag="gj")
                    nc.gpsimd.dma_gather(
                        out_ap=gj[:].rearrange("p (s e) -> p s e", e=128),
                        in_ap=zj_tab,
                        idxs_ap=jx_sb[:, cb:cb + S // 16],
                        num_idxs=S,
                        num_idxs_reg=size_regs[S],
                        elem_size=128,
                    )
                    gi = gp.tile([128, S], f32, tag="gi")
                    nc.gpsimd.dma_gather(
                        out_ap=gi[:].rearrange("p (s e) -> p s e", e=128),
                        in_ap=zi_tab,
                        idxs_ap=ix_sb[:, cb:cb + S // 16],
                        num_idxs=S,
                        num_idxs_reg=size_regs[S],
                        elem_size=128,
                    )

                    logits = accp.tile([128, CHUNK // 128], f32, tag="logits")
                    for grp in range(slots // GRP):
                        tp = pst.tile([128, GRP * D], f32)
                        for u in range(GRP):
                            t = grp * GRP + u
                            nc.tensor.transpose(
                                tp[:, u * D:(u + 1) * D],
                                gj[:, t * D:(t + 1) * D],
                                ident[:],
                            )
                        tps = zjtp.tile([128, GRP * D], f32)
                        nc.scalar.copy(tps[:], tp[:])

                        mm = psm.tile([128, GRP * D], f32)
                        for u in range(GRP):
                            nc.tensor.matmul(
                                mm[:, u * D:(u + 1) * D],
                                lhsT=tps[:, u * D:(u + 1) * D],
                                rhs=w_sb[:],
                                start=True,
                                stop=True,
                            )
                        mms = mmsp.tile([128, GRP * D], f32)
                        nc.scalar.copy(mms[:], mm[:])

                        prod = prodp.tile([128, GRP * D], f32)
                        zi = gi[:, grp * GRP * D:(grp + 1) * GRP * D]
                        nc.vector.tensor_mul(out=prod[:], in0=zi, in1=mms[:])
                        nc.vector.reduce_sum(
                            out=logits[:, grp * GRP:(grp + 1) * GRP],
                            in_=prod[:].rearrange("p (u f) -> p u f", f=D),
                            axis=mybir.AxisListType.X,
                        )

                    probs = accp.tile([128, CHUNK // 128], f32, tag="probs")
                    nc.scalar.activation(
                        probs[:, :slots], logits[:, :slots],
                        mybir.ActivationFunctionType.Sigmoid,
                    )
                    nc.sync.dma_start(
                        out_d[:, base // 128:base // 128 + slots],
                        probs[:, :slots],
                    )
                    base += S

    return nc


def _get_program(caps, split):
    import concourse.mybir as mybir

    key = (tuple(caps), split)
    if key not in _CACHE:
        nc = _build_program(tuple(caps))
        if split:
            _split_multi_waits(nc)
            # populate .instr bytes for InstISA subclasses (the library
            # reload); without this walrus fails with "ISA wrong length"
            mybir.codegen_inst_isa_subclasses(nc)
        _CACHE[key] = nc
    return _CACHE[key]


def _preprocess(z, edge_index, W):
    """Classify/pad edges per core; build per-core device inputs and the
    inverse mapping. Returns (caps, in_maps, perms)."""
    z = np.ascontiguousarray(np.asarray(z, dtype=np.float32))
    W = np.ascontiguousarray(np.asarray(W, dtype=np.float32))
    ident = np.eye(D, dtype=np.float32)
    ei = np.asarray(edge_index).astype(np.int64)
    jj_all = ei[1]
    ii_all = ei[0]

    per_core = []
    counts = np.zeros((NCORES, NCLS), np.int64)
    for c in range(NCORES):
        sl = slice(c * EPC, (c + 1) * EPC)
        jj = jj_all[sl]
        ii = ii_all[sl]
        cls = (jj // SUBROWS) * NSUB + (ii // SUBROWS)
        order = np.argsort(cls, kind="stable")
        counts[c] = np.bincount(cls, minlength=NCLS)
        per_core.append((jj, ii, cls, order))

    caps = counts.max(axis=0)
    caps = ((caps + PADQ - 1) // PADQ) * PADQ
    caps = tuple(int(x) for x in caps)
    tot = sum(caps)

    in_maps = []
    perms = []
    for c in range(NCORES):
        jj, ii, cls, order = per_core[c]
        j16 = np.zeros(tot, np.int16)
        i16 = np.zeros(tot, np.int16)
        perm = np.full(tot, -1, np.int64)
        base = 0
        cnt = counts[c]
        cstart = np.zeros(NCLS + 1, np.int64)
        cstart[1:] = np.cumsum(cnt)
        for k in range(NCLS):
            ids = order[cstart[k]:cstart[k + 1]]
            n = len(ids)
            bj, bi = divmod(k, NSUB)
            j16[base:base + n] = (jj[ids] - bj * SUBROWS).astype(np.int16)
            i16[base:base + n] = (ii[ids] - bi * SUBROWS).astype(np.int16)
            perm[base:base + n] = ids
            base += caps[k]
        # wrap into [16, tot/16] (position q -> [q%16, q//16]), replicate x8
        jw = np.tile(j16.reshape(-1, 16).T, (8, 1)).astype(np.int16)
        iw = np.tile(i16.reshape(-1, 16).T, (8, 1)).astype(np.int16)
        in_maps.append({
            "z": z, "w": W, "ident": ident,
            "jx": np.ascontiguousarray(jw),
            "ix": np.ascontiguousarray(iw),
        })
        perms.append(perm)
    return caps, in_maps, perms


def kernel(z, edge_index, W):
    from concourse.bass_utils import run_bass_kernel_spmd

    caps, in_maps, perms = _preprocess(z, edge_index, W)
    nc = _get_program(caps, split=True)
    res = run_bass_kernel_spmd(nc, in_maps, core_ids=list(range(NCORES)))
    out = np.empty(E, np.float32)
    for c in range(NCORES):
        o = res.results[c]["out"]          # [128, tot_slots]
        padded = o.T.ravel()               # padded position q = slot*128 + p
        perm = perms[c]
        valid = perm >= 0
        core_out = np.empty(EPC, np.float32)
        core_out[perm[valid]] = padded[valid]
        out[c * EPC:(c + 1) * EPC] = core_out
    return out


# revision 22
# speedup vs baseline: 1.8162x; 1.8162x over previous
"""Bilinear edge decoder on 8 TRN2 NeuronCores.

out[e] = sigmoid( z[ei[1,e]] @ W @ z[ei[0,e]] )  for e in [0, 600000)

Strategy (edge-sharded data parallel, dma_gather):
  - Split the 600k edges evenly across 8 cores (75k each); replicate z, W.
  - The HW gather primitive (InstDMAGatherAnt, GPSIMD 'mlp' library) uses
    int16 row indices, so z is treated as 4 sub-tables of 25000 rows.
    Each core's edges are classified into 16 classes by the pair
    (j_subtable, i_subtable) so that, within a class, both endpoint
    gathers use a single sub-table each and share one slot order.
  - Classes are padded (with row-0 dummy edges) to shared static
    capacities (max over cores, rounded to 512) so all 8 cores run one
    SPMD program; the capacities come from the actual input, and the
    program is compiled per capacity signature inside kernel().
  - Per chunk (<=4096 edges) of a class: two dma_gathers fetch z_j and
    z_i rows into [128, slots, 128] tiles (edge q -> partition q%128,
    slot q//128). Per 128-edge slot: PE-transpose z_j, matmul with W,
    DVE multiply with z_i and reduce over features, ACT sigmoid, DMA out.
  - Walrus codegen allows a single sync wait per TPB instruction;
    _split_multi_waits legalizes the Tile-scheduled program by splitting
    extra waits into standalone InstEventSemaphore ops.
"""

import numpy as np

N_NODES = 100000
D = 128
E = 600000
NCORES = 8
EPC = E // NCORES           # 75000 edges per core
NSUB = 4
SUBROWS = 25000             # fits int16 index range
NCLS = NSUB * NSUB
CHUNK = 1024                # max edges per gather op (Q7 gather limit ~1024 idx)
DMA_SCRATCH = 16384         # SWDGE descriptor ring bytes (1024 descs per 16KB)
NQUEUES = 4                 # SWDGE queues; each gather runs on Q7 cpu pair queue_num
PADQ = 512                  # class capacity quantum (keeps 4-slot groups even)
GRP = 4                     # slots batched per PSUM bank (512 f32)

_CACHE = {}


def _split_multi_waits(nc):
    """Walrus codegen allows at most one sync wait per TPB instruction.
    Split any instruction with multiple sem-ge waits into preceding
    single-wait InstEventSemaphore ops on the same engine."""
    import concourse.mybir as mybir

    n = 0
    for f in nc.m.functions:
        for blk in f.blocks:
            new = []
            for inst in blk.instructions:
                si = inst.sync_info
                if (
                    si is not None
                    and si.on_wait
                    and len(si.on_wait) > 1
                    and all(
                        w.wait_mode == "sem-ge-imm" and w.wait_reg is None
                        for w in si.on_wait
                    )
                ):
                    waits = list(si.on_wait)
                    for w in waits[:-1]:
                        ev = mybir.InstEventSemaphore(
                            name=f"EVSPLIT-{n}", ins=[], outs=[]
                        )
                        n += 1
                        ev.engine = inst.engine
                        ev.sync_info = mybir.SyncInfo(on_wait=[w], on_update=[])
                        new.append(ev)
                    inst.sync_info = mybir.SyncInfo(
                        on_wait=[waits[-1]], on_update=list(si.on_update)
                    )
                new.append(inst)
            blk.instructions = new
    return n


def _fix_gather_queues(nc):
    """Tile assigns DMASW sem lanes round-robin in *scheduled* order, and the
    runtime locks each lane to one SWDGE queue. Derive queue_num from the
    assigned lane so they always agree."""
    import concourse.mybir as mybir

    for f in nc.m.functions:
        for blk in f.blocks:
            for inst in blk.instructions:
                if type(inst).__name__ == "InstDMAGatherAnt":
                    si = inst.sync_info
                    assert si and si.on_update, inst
                    name = si.on_update[0].ant_name  # e.g. DMASW3_44
                    assert name.startswith("DMASW"), name
                    lane = int(name[5:].split("_")[0])
                    inst.queue_num = lane % NQUEUES


def _chunks_of(cap):
    out = []
    left = cap
    while left > 0:
        s = min(CHUNK, left)
        out.append(s)
        left -= s
    return out


def _build_program(caps):
    import concourse.bass as bass
    import concourse.mybir as mybir
    import concourse.tile as tile
    from concourse import library_config

    f32 = mybir.dt.float32
    i16 = mybir.dt.int16

    tot = sum(caps)
    tot_slots = tot // 128
    idx_cols = tot // 16

    nc = bass.Bass("TRN2", target_bir_lowering=False, debug=False,
                   num_devices=NCORES,
                   dynamic_dma_scratch_size=DMA_SCRATCH,
                   num_swdge_queues=NQUEUES)

    z_d = nc.dram_tensor("z", [N_NODES, D], f32, kind="ExternalInput")
    w_d = nc.dram_tensor("w", [D, D], f32, kind="ExternalInput")
    id_d = nc.dram_tensor("ident", [D, D], f32, kind="ExternalInput")
    jx_d = nc.dram_tensor("jx", [128, idx_cols], i16, kind="ExternalInput")
    ix_d = nc.dram_tensor("ix", [128, idx_cols], i16, kind="ExternalInput")
    out_d = nc.dram_tensor("out", [128, tot_slots], f32, kind="ExternalOutput")

    with tile.TileContext(nc) as tc:
        with (
            tc.tile_pool(name="const", bufs=1) as constp,
            tc.tile_pool(name="g", bufs=2) as gp,
            tc.tile_pool(name="zjt", bufs=3) as zjtp,
            tc.tile_pool(name="mms", bufs=3) as mmsp,
            tc.tile_pool(name="prod", bufs=3) as prodp,
            tc.tile_pool(name="acc", bufs=2) as accp,
            tc.tile_pool(name="scr", bufs=1, space="PSUM") as scrp,
            tc.tile_pool(name="pst", bufs=3, space="PSUM") as pst,
            tc.tile_pool(name="psm", bufs=3, space="PSUM") as psm,
        ):
            nc.gpsimd.load_library(library_config.mlp)

            ident = constp.tile([128, 128], f32)
            nc.sync.dma_start(ident[:], id_d[:, :])
            w_sb = constp.tile([128, 128], f32)
            nc.sync.dma_start(w_sb[:], w_d[:, :])
            jx_sb = constp.tile([128, idx_cols], i16)
            nc.sync.dma_start(jx_sb[:], jx_d[:, :])
            ix_sb = constp.tile([128, idx_cols], i16)
            nc.sync.dma_start(ix_sb[:], ix_d[:, :])

            # dummy PE ops: absorb the constant-load DMA waits once
            scr = scrp.tile([128, 128], f32)
            nc.tensor.transpose(scr[:], ident[:], ident[:])
            scr2 = scrp.tile([128, 128], f32, tag="scr2")
            nc.tensor.matmul(scr2[:], lhsT=w_sb[:], rhs=w_sb[:],
                             start=True, stop=True)

            # one shared register per distinct chunk size (Pool registers
            # are scarce; to_reg per gather would exhaust them)
            size_regs = {}
            for cls in range(NCLS):
                for S in _chunks_of(caps[cls]):
                    if S not in size_regs:
                        size_regs[S] = nc.gpsimd.to_reg(S)

            base = 0
            qrr = [0]
            for cls in range(NCLS):
                bj, bi = divmod(cls, NSUB)
                zj_tab = z_d[bj * SUBROWS:(bj + 1) * SUBROWS, :]
                zi_tab = z_d[bi * SUBROWS:(bi + 1) * SUBROWS, :]
                for S in _chunks_of(caps[cls]):
                    slots = S // 128
                    cb = base // 16
                    gj = gp.tile([128, S], f32, tag="gj")
                    nc.gpsimd.dma_gather(
                        out_ap=gj[:].rearrange("p (s e) -> p s e", e=128),
                        in_ap=zj_tab,
                        idxs_ap=jx_sb[:, cb:cb + S // 16],
                        num_idxs=S,
                        num_idxs_reg=size_regs[S],
                        elem_size=128,
                        queue_num=0,
                    )
                    qrr[0] += 1
                    gi = gp.tile([128, S], f32, tag="gi")
                    nc.gpsimd.dma_gather(
                        out_ap=gi[:].rearrange("p (s e) -> p s e", e=128),
                        in_ap=zi_tab,
                        idxs_ap=ix_sb[:, cb:cb + S // 16],
                        num_idxs=S,
                        num_idxs_reg=size_regs[S],
                        elem_size=128,
                        queue_num=0,
                    )
                    qrr[0] += 1

                    logits = accp.tile([128, CHUNK // 128], f32, tag="logits")
                    for grp in range(slots // GRP):
                        tp = pst.tile([128, GRP * D], f32)
                        for u in range(GRP):
                            t = grp * GRP + u
                            nc.tensor.transpose(
                                tp[:, u * D:(u + 1) * D],
                                gj[:, t * D:(t + 1) * D],
                                ident[:],
                            )
                        tps = zjtp.tile([128, GRP * D], f32)
                        nc.scalar.copy(tps[:], tp[:])

                        mm = psm.tile([128, GRP * D], f32)
                        for u in range(GRP):
                            nc.tensor.matmul(
                                mm[:, u * D:(u + 1) * D],
                                lhsT=tps[:, u * D:(u + 1) * D],
                                rhs=w_sb[:],
                                start=True,
                                stop=True,
                            )
                        mms = mmsp.tile([128, GRP * D], f32)
                        nc.scalar.copy(mms[:], mm[:])

                        prod = prodp.tile([128, GRP * D], f32)
                        zi = gi[:, grp * GRP * D:(grp + 1) * GRP * D]
                        nc.vector.tensor_mul(out=prod[:], in0=zi, in1=mms[:])
                        nc.vector.reduce_sum(
                            out=logits[:, grp * GRP:(grp + 1) * GRP],
                            in_=prod[:].rearrange("p (u f) -> p u f", f=D),
                            axis=mybir.AxisListType.X,
                        )

                    probs = accp.tile([128, CHUNK // 128], f32, tag="probs")
                    nc.scalar.activation(
                        probs[:, :slots], logits[:, :slots],
                        mybir.ActivationFunctionType.Sigmoid,
                    )
                    nc.sync.dma_start(
                        out_d[:, base // 128:base // 128 + slots],
                        probs[:, :slots],
                    )
                    base += S

    return nc


def _get_program(caps, split):
    import concourse.mybir as mybir

    key = (tuple(caps), split)
    if key not in _CACHE:
        nc = _build_program(tuple(caps))
        _fix_gather_queues(nc)
        if split:
            _split_multi_waits(nc)
            # populate .instr bytes for InstISA subclasses (the library
            # reload); without this walrus fails with "ISA wrong length"
            mybir.codegen_inst_isa_subclasses(nc)
        _CACHE[key] = nc
    return _CACHE[key]


def _preprocess(z, edge_index, W):
    """Classify/pad edges per core; build per-core device inputs and the
    inverse mapping. Returns (caps, in_maps, perms)."""
    z = np.ascontiguousarray(np.asarray(z, dtype=np.float32))
    W = np.ascontiguousarray(np.asarray(W, dtype=np.float32))
    ident = np.eye(D, dtype=np.float32)
    ei = np.asarray(edge_index).astype(np.int64)
    jj_all = ei[1]
    ii_all = ei[0]

    per_core = []
    counts = np.zeros((NCORES, NCLS), np.int64)
    for c in range(NCORES):
        sl = slice(c * EPC, (c + 1) * EPC)
        jj = jj_all[sl]
        ii = ii_all[sl]
        cls = (jj // SUBROWS) * NSUB + (ii // SUBROWS)
        order = np.argsort(cls, kind="stable")
        counts[c] = np.bincount(cls, minlength=NCLS)
        per_core.append((jj, ii, cls, order))

    caps = counts.max(axis=0)
    caps = ((caps + PADQ - 1) // PADQ) * PADQ
    caps = tuple(int(x) for x in caps)
    tot = sum(caps)

    in_maps = []
    perms = []
    for c in range(NCORES):
        jj, ii, cls, order = per_core[c]
        j16 = np.zeros(tot, np.int16)
        i16 = np.zeros(tot, np.int16)
        perm = np.full(tot, -1, np.int64)
        base = 0
        cnt = counts[c]
        cstart = np.zeros(NCLS + 1, np.int64)
        cstart[1:] = np.cumsum(cnt)
        for k in range(NCLS):
            ids = order[cstart[k]:cstart[k + 1]]
            n = len(ids)
            bj, bi = divmod(k, NSUB)
            j16[base:base + n] = (jj[ids] - bj * SUBROWS).astype(np.int16)
            i16[base:base + n] = (ii[ids] - bi * SUBROWS).astype(np.int16)
            perm[base:base + n] = ids
            base += caps[k]
        # wrap into [16, tot/16] (position q -> [q%16, q//16]), replicate x8
        jw = np.tile(j16.reshape(-1, 16).T, (8, 1)).astype(np.int16)
        iw = np.tile(i16.reshape(-1, 16).T, (8, 1)).astype(np.int16)
        in_maps.append({
            "z": z, "w": W, "ident": ident,
            "jx": np.ascontiguousarray(jw),
            "ix": np.ascontiguousarray(iw),
        })
        perms.append(perm)
    return caps, in_maps, perms


def kernel(z, edge_index, W):
    from concourse.bass_utils import run_bass_kernel_spmd

    caps, in_maps, perms = _preprocess(z, edge_index, W)
    nc = _get_program(caps, split=True)
    res = run_bass_kernel_spmd(nc, in_maps, core_ids=list(range(NCORES)))
    out = np.empty(E, np.float32)
    for c in range(NCORES):
        o = res.results[c]["out"]          # [128, tot_slots]
        padded = o.T.ravel()               # padded position q = slot*128 + p
        perm = perms[c]
        valid = perm >= 0
        core_out = np.empty(EPC, np.float32)
        core_out[perm[valid]] = padded[valid]
        out[c * EPC:(c + 1) * EPC] = core_out
    return out


# revision 23
# speedup vs baseline: 2.8304x; 1.5584x over previous
"""Bilinear edge decoder on 8 TRN2 NeuronCores.

out[e] = sigmoid( z[ei[1,e]] @ W @ z[ei[0,e]] )  for e in [0, 600000)

Strategy (edge-sharded data parallel, dma_gather):
  - Split the 600k edges evenly across 8 cores (75k each); replicate z, W.
  - The HW gather primitive (InstDMAGatherAnt, GPSIMD 'mlp' library) uses
    int16 row indices, so z is treated as 4 sub-tables of 25000 rows.
    Each core's edges are classified into 16 classes by the pair
    (j_subtable, i_subtable) so that, within a class, both endpoint
    gathers use a single sub-table each and share one slot order.
  - Classes are padded (with row-0 dummy edges) to shared static
    capacities (max over cores, rounded to 512) so all 8 cores run one
    SPMD program; the capacities come from the actual input, and the
    program is compiled per capacity signature inside kernel().
  - Per chunk (<=4096 edges) of a class: two dma_gathers fetch z_j and
    z_i rows into [128, slots, 128] tiles (edge q -> partition q%128,
    slot q//128). Per 128-edge slot: PE-transpose z_j, matmul with W,
    DVE multiply with z_i and reduce over features, ACT sigmoid, DMA out.
  - Walrus codegen allows a single sync wait per TPB instruction;
    _split_multi_waits legalizes the Tile-scheduled program by splitting
    extra waits into standalone InstEventSemaphore ops.
"""

import numpy as np

N_NODES = 100000
D = 128
E = 600000
NCORES = 8
EPC = E // NCORES           # 75000 edges per core
NSUB = 4
SUBROWS = 25000             # fits int16 index range
NCLS = NSUB * NSUB
CHUNK = 1024                # max edges per gather op (Q7 gather limit ~1024 idx)
DMA_SCRATCH = 16384         # SWDGE descriptor ring bytes (1024 descs per 16KB)
NQUEUES = 4                 # SWDGE queues; each gather runs on Q7 cpu pair queue_num
PADQ = 512                  # class capacity quantum (keeps 4-slot groups even)
GRP = 4                     # slots batched per PSUM bank (512 f32)

_CACHE = {}


def _split_multi_waits(nc):
    """Walrus codegen allows at most one sync wait per TPB instruction.
    Split any instruction with multiple sem-ge waits into preceding
    single-wait InstEventSemaphore ops on the same engine."""
    import concourse.mybir as mybir

    n = 0
    for f in nc.m.functions:
        for blk in f.blocks:
            new = []
            for inst in blk.instructions:
                si = inst.sync_info
                if (
                    si is not None
                    and si.on_wait
                    and len(si.on_wait) > 1
                    and all(
                        w.wait_mode == "sem-ge-imm" and w.wait_reg is None
                        for w in si.on_wait
                    )
                ):
                    waits = list(si.on_wait)
                    for w in waits[:-1]:
                        ev = mybir.InstEventSemaphore(
                            name=f"EVSPLIT-{n}", ins=[], outs=[]
                        )
                        n += 1
                        ev.engine = inst.engine
                        ev.sync_info = mybir.SyncInfo(on_wait=[w], on_update=[])
                        new.append(ev)
                    inst.sync_info = mybir.SyncInfo(
                        on_wait=[waits[-1]], on_update=list(si.on_update)
                    )
                new.append(inst)
            blk.instructions = new
    return n


def _fix_gather_queues(nc):
    """Tile assigns DMASW sem lanes round-robin in *scheduled* order, and the
    runtime locks each lane to one SWDGE queue. Derive queue_num from the
    assigned lane so they always agree."""
    import concourse.mybir as mybir

    for f in nc.m.functions:
        for blk in f.blocks:
            for inst in blk.instructions:
                if type(inst).__name__ == "InstDMAGatherAnt":
                    si = inst.sync_info
                    assert si and si.on_update, inst
                    name = si.on_update[0].ant_name  # e.g. DMASW3_44
                    assert name.startswith("DMASW"), name
                    lane = int(name[5:].split("_")[0])
                    inst.queue_num = lane % NQUEUES


def _chunks_of(cap):
    out = []
    left = cap
    while left > 0:
        s = min(CHUNK, left)
        out.append(s)
        left -= s
    return out


def _build_program(caps):
    import concourse.bass as bass
    import concourse.mybir as mybir
    import concourse.tile as tile
    from concourse import library_config

    f32 = mybir.dt.float32
    i16 = mybir.dt.int16

    tot = sum(caps)
    tot_slots = tot // 128
    idx_cols = tot // 16

    nc = bass.Bass("TRN2", target_bir_lowering=False, debug=False,
                   num_devices=NCORES,
                   dynamic_dma_scratch_size=DMA_SCRATCH,
                   num_swdge_queues=NQUEUES)

    z_d = nc.dram_tensor("z", [N_NODES, D], f32, kind="ExternalInput")
    w_d = nc.dram_tensor("w", [D, D], f32, kind="ExternalInput")
    id_d = nc.dram_tensor("ident", [D, D], f32, kind="ExternalInput")
    jx_d = nc.dram_tensor("jx", [128, idx_cols], i16, kind="ExternalInput")
    ix_d = nc.dram_tensor("ix", [128, idx_cols], i16, kind="ExternalInput")
    out_d = nc.dram_tensor("out", [128, tot_slots], f32, kind="ExternalOutput")

    with tile.TileContext(nc) as tc:
        with (
            tc.tile_pool(name="const", bufs=1) as constp,
            tc.tile_pool(name="g", bufs=6) as gp,
            tc.tile_pool(name="zjt", bufs=3) as zjtp,
            tc.tile_pool(name="mms", bufs=3) as mmsp,
            tc.tile_pool(name="prod", bufs=3) as prodp,
            tc.tile_pool(name="acc", bufs=2) as accp,
            tc.tile_pool(name="scr", bufs=1, space="PSUM") as scrp,
            tc.tile_pool(name="pst", bufs=3, space="PSUM") as pst,
            tc.tile_pool(name="psm", bufs=3, space="PSUM") as psm,
        ):
            nc.gpsimd.load_library(library_config.mlp)

            ident = constp.tile([128, 128], f32)
            nc.sync.dma_start(ident[:], id_d[:, :])
            w_sb = constp.tile([128, 128], f32)
            nc.sync.dma_start(w_sb[:], w_d[:, :])
            jx_sb = constp.tile([128, idx_cols], i16)
            nc.sync.dma_start(jx_sb[:], jx_d[:, :])
            ix_sb = constp.tile([128, idx_cols], i16)
            nc.sync.dma_start(ix_sb[:], ix_d[:, :])

            # dummy PE ops: absorb the constant-load DMA waits once
            scr = scrp.tile([128, 128], f32)
            nc.tensor.transpose(scr[:], ident[:], ident[:])
            scr2 = scrp.tile([128, 128], f32, tag="scr2")
            nc.tensor.matmul(scr2[:], lhsT=w_sb[:], rhs=w_sb[:],
                             start=True, stop=True)

            # one shared register per distinct chunk size (Pool registers
            # are scarce; to_reg per gather would exhaust them)
            size_regs = {}
            for cls in range(NCLS):
                for S in _chunks_of(caps[cls]):
                    if S not in size_regs:
                        size_regs[S] = nc.gpsimd.to_reg(S)

            base = 0
            qrr = [0]
            for cls in range(NCLS):
                bj, bi = divmod(cls, NSUB)
                zj_tab = z_d[bj * SUBROWS:(bj + 1) * SUBROWS, :]
                zi_tab = z_d[bi * SUBROWS:(bi + 1) * SUBROWS, :]
                for S in _chunks_of(caps[cls]):
                    slots = S // 128
                    cb = base // 16
                    gj = gp.tile([128, S], f32, tag="gj")
                    nc.gpsimd.dma_gather(
                        out_ap=gj[:].rearrange("p (s e) -> p s e", e=128),
                        in_ap=zj_tab,
                        idxs_ap=jx_sb[:, cb:cb + S // 16],
                        num_idxs=S,
                        num_idxs_reg=size_regs[S],
                        elem_size=128,
                        queue_num=0,
                    )
                    qrr[0] += 1
                    gi = gp.tile([128, S], f32, tag="gi")
                    nc.gpsimd.dma_gather(
                        out_ap=gi[:].rearrange("p (s e) -> p s e", e=128),
                        in_ap=zi_tab,
                        idxs_ap=ix_sb[:, cb:cb + S // 16],
                        num_idxs=S,
                        num_idxs_reg=size_regs[S],
                        elem_size=128,
                        queue_num=0,
                    )
                    qrr[0] += 1

                    logits = accp.tile([128, CHUNK // 128], f32, tag="logits")
                    for grp in range(slots // GRP):
                        tp = pst.tile([128, GRP * D], f32)
                        for u in range(GRP):
                            t = grp * GRP + u
                            nc.tensor.transpose(
                                tp[:, u * D:(u + 1) * D],
                                gj[:, t * D:(t + 1) * D],
                                ident[:],
                            )
                        tps = zjtp.tile([128, GRP * D], f32)
                        nc.scalar.copy(tps[:], tp[:])

                        mm = psm.tile([128, GRP * D], f32)
                        for u in range(GRP):
                            nc.tensor.matmul(
                                mm[:, u * D:(u + 1) * D],
                                lhsT=tps[:, u * D:(u + 1) * D],
                                rhs=w_sb[:],
                                start=True,
                                stop=True,
                            )
                        mms = mmsp.tile([128, GRP * D], f32)
                        nc.scalar.copy(mms[:], mm[:])

                        prod = prodp.tile([128, GRP * D], f32)
                        zi = gi[:, grp * GRP * D:(grp + 1) * GRP * D]
                        nc.vector.tensor_mul(out=prod[:], in0=zi, in1=mms[:])
                        nc.vector.reduce_sum(
                            out=logits[:, grp * GRP:(grp + 1) * GRP],
                            in_=prod[:].rearrange("p (u f) -> p u f", f=D),
                            axis=mybir.AxisListType.X,
                        )

                    probs = accp.tile([128, CHUNK // 128], f32, tag="probs")
                    nc.scalar.activation(
                        probs[:, :slots], logits[:, :slots],
                        mybir.ActivationFunctionType.Sigmoid,
                    )
                    nc.sync.dma_start(
                        out_d[:, base // 128:base // 128 + slots],
                        probs[:, :slots],
                    )
                    base += S

    return nc


def _get_program(caps, split):
    import concourse.mybir as mybir

    key = (tuple(caps), split)
    if key not in _CACHE:
        nc = _build_program(tuple(caps))
        _fix_gather_queues(nc)
        if split:
            _split_multi_waits(nc)
            # populate .instr bytes for InstISA subclasses (the library
            # reload); without this walrus fails with "ISA wrong length"
            mybir.codegen_inst_isa_subclasses(nc)
        _CACHE[key] = nc
    return _CACHE[key]


def _preprocess(z, edge_index, W):
    """Classify/pad edges per core; build per-core device inputs and the
    inverse mapping. Returns (caps, in_maps, perms)."""
    z = np.ascontiguousarray(np.asarray(z, dtype=np.float32))
    W = np.ascontiguousarray(np.asarray(W, dtype=np.float32))
    ident = np.eye(D, dtype=np.float32)
    ei = np.asarray(edge_index).astype(np.int64)
    jj_all = ei[1]
    ii_all = ei[0]

    per_core = []
    counts = np.zeros((NCORES, NCLS), np.int64)
    for c in range(NCORES):
        sl = slice(c * EPC, (c + 1) * EPC)
        jj = jj_all[sl]
        ii = ii_all[sl]
        cls = (jj // SUBROWS) * NSUB + (ii // SUBROWS)
        order = np.argsort(cls, kind="stable")
        counts[c] = np.bincount(cls, minlength=NCLS)
        per_core.append((jj, ii, cls, order))

    caps = counts.max(axis=0)
    caps = ((caps + PADQ - 1) // PADQ) * PADQ
    caps = tuple(int(x) for x in caps)
    tot = sum(caps)

    in_maps = []
    perms = []
    for c in range(NCORES):
        jj, ii, cls, order = per_core[c]
        j16 = np.zeros(tot, np.int16)
        i16 = np.zeros(tot, np.int16)
        perm = np.full(tot, -1, np.int64)
        base = 0
        cnt = counts[c]
        cstart = np.zeros(NCLS + 1, np.int64)
        cstart[1:] = np.cumsum(cnt)
        for k in range(NCLS):
            ids = order[cstart[k]:cstart[k + 1]]
            n = len(ids)
            bj, bi = divmod(k, NSUB)
            j16[base:base + n] = (jj[ids] - bj * SUBROWS).astype(np.int16)
            i16[base:base + n] = (ii[ids] - bi * SUBROWS).astype(np.int16)
            perm[base:base + n] = ids
            base += caps[k]
        # wrap into [16, tot/16] (position q -> [q%16, q//16]), replicate x8
        jw = np.tile(j16.reshape(-1, 16).T, (8, 1)).astype(np.int16)
        iw = np.tile(i16.reshape(-1, 16).T, (8, 1)).astype(np.int16)
        in_maps.append({
            "z": z, "w": W, "ident": ident,
            "jx": np.ascontiguousarray(jw),
            "ix": np.ascontiguousarray(iw),
        })
        perms.append(perm)
    return caps, in_maps, perms


def kernel(z, edge_index, W):
    from concourse.bass_utils import run_bass_kernel_spmd

    caps, in_maps, perms = _preprocess(z, edge_index, W)
    nc = _get_program(caps, split=True)
    res = run_bass_kernel_spmd(nc, in_maps, core_ids=list(range(NCORES)))
    out = np.empty(E, np.float32)
    for c in range(NCORES):
        o = res.results[c]["out"]          # [128, tot_slots]
        padded = o.T.ravel()               # padded position q = slot*128 + p
        perm = perms[c]
        valid = perm >= 0
        core_out = np.empty(EPC, np.float32)
        core_out[perm[valid]] = padded[valid]
        out[c * EPC:(c + 1) * EPC] = core_out
    return out


# revision 25
# speedup vs baseline: 3.4315x; 1.2124x over previous
"""Bilinear edge decoder on 8 TRN2 NeuronCores.

out[e] = sigmoid( z[ei[1,e]] @ W @ z[ei[0,e]] )  for e in [0, 600000)

Strategy (edge-sharded data parallel, dma_gather):
  - Split the 600k edges evenly across 8 cores (75k each); replicate z, W.
  - The HW gather primitive (InstDMAGatherAnt, GPSIMD 'mlp' library) uses
    int16 row indices, so z is treated as 4 sub-tables of 25000 rows.
    Each core's edges are classified into 16 classes by the pair
    (j_subtable, i_subtable) so that, within a class, both endpoint
    gathers use a single sub-table each and share one slot order.
  - Classes are padded (with row-0 dummy edges) to shared static
    capacities (max over cores, rounded to 512) so all 8 cores run one
    SPMD program; the capacities come from the actual input, and the
    program is compiled per capacity signature inside kernel().
  - Per chunk (<=4096 edges) of a class: two dma_gathers fetch z_j and
    z_i rows into [128, slots, 128] tiles (edge q -> partition q%128,
    slot q//128). Per 128-edge slot: PE-transpose z_j, matmul with W,
    DVE multiply with z_i and reduce over features, ACT sigmoid, DMA out.
  - Walrus codegen allows a single sync wait per TPB instruction;
    _split_multi_waits legalizes the Tile-scheduled program by splitting
    extra waits into standalone InstEventSemaphore ops.
"""

import numpy as np

N_NODES = 100000
D = 128
E = 600000
NCORES = 8
EPC = E // NCORES           # 75000 edges per core
NSUB = 4
SUBROWS = 25000             # fits int16 index range
NCLS = NSUB * NSUB
CHUNK = 1024                # max edges per gather op (Q7 gather limit ~1024 idx)
DMA_SCRATCH = 16384         # SWDGE descriptor ring bytes (1024 descs per 16KB)
NQUEUES = 4                 # SWDGE queues; each gather runs on Q7 cpu pair queue_num
PADQ = 512                  # class capacity quantum (keeps 4-slot groups even)
GRP = 4                     # slots batched per PSUM bank (512 f32)

_CACHE = {}


def _split_multi_waits(nc):
    """Walrus codegen allows at most one sync wait per TPB instruction.
    Split any instruction with multiple sem-ge waits into preceding
    single-wait InstEventSemaphore ops on the same engine."""
    import concourse.mybir as mybir

    n = 0
    for f in nc.m.functions:
        for blk in f.blocks:
            new = []
            for inst in blk.instructions:
                si = inst.sync_info
                if (
                    si is not None
                    and si.on_wait
                    and len(si.on_wait) > 1
                    and all(
                        w.wait_mode == "sem-ge-imm" and w.wait_reg is None
                        for w in si.on_wait
                    )
                ):
                    waits = list(si.on_wait)
                    for w in waits[:-1]:
                        ev = mybir.InstEventSemaphore(
                            name=f"EVSPLIT-{n}", ins=[], outs=[]
                        )
                        n += 1
                        ev.engine = inst.engine
                        ev.sync_info = mybir.SyncInfo(on_wait=[w], on_update=[])
                        new.append(ev)
                    inst.sync_info = mybir.SyncInfo(
                        on_wait=[waits[-1]], on_update=list(si.on_update)
                    )
                new.append(inst)
            blk.instructions = new
    return n


def _fix_gather_queues(nc):
    """Tile assigns DMASW sem lanes round-robin in *scheduled* order, and the
    runtime locks each lane to one SWDGE queue. Derive queue_num from the
    assigned lane so they always agree."""
    import concourse.mybir as mybir

    for f in nc.m.functions:
        for blk in f.blocks:
            for inst in blk.instructions:
                if type(inst).__name__ == "InstDMAGatherAnt":
                    si = inst.sync_info
                    assert si and si.on_update, inst
                    name = si.on_update[0].ant_name  # e.g. DMASW3_44
                    assert name.startswith("DMASW"), name
                    lane = int(name[5:].split("_")[0])
                    inst.queue_num = lane % NQUEUES


def _chunks_of(cap):
    out = []
    left = cap
    while left > 0:
        s = min(CHUNK, left)
        out.append(s)
        left -= s
    return out


def _build_program(caps):
    import concourse.bass as bass
    import concourse.mybir as mybir
    import concourse.tile as tile
    from concourse import library_config

    f32 = mybir.dt.float32
    i16 = mybir.dt.int16

    tot = sum(caps)
    tot_slots = tot // 128
    idx_cols = tot // 16

    nc = bass.Bass("TRN2", target_bir_lowering=False, debug=False,
                   num_devices=NCORES,
                   dynamic_dma_scratch_size=DMA_SCRATCH,
                   num_swdge_queues=NQUEUES)

    z_d = nc.dram_tensor("z", [N_NODES, D], f32, kind="ExternalInput")
    w_d = nc.dram_tensor("w", [D, D], f32, kind="ExternalInput")
    id_d = nc.dram_tensor("ident", [D, D], f32, kind="ExternalInput")
    jx_d = nc.dram_tensor("jx", [128, idx_cols], i16, kind="ExternalInput")
    ix_d = nc.dram_tensor("ix", [128, idx_cols], i16, kind="ExternalInput")
    nchunks = sum(len(_chunks_of(c)) for c in caps)
    cnt_d = nc.dram_tensor("cnt", [1, nchunks], mybir.dt.int32,
                           kind="ExternalInput")
    out_d = nc.dram_tensor("out", [128, tot_slots], f32, kind="ExternalOutput")

    with tile.TileContext(nc) as tc:
        with (
            tc.tile_pool(name="const", bufs=1) as constp,
            tc.tile_pool(name="g", bufs=8) as gp,
            tc.tile_pool(name="zjt", bufs=3) as zjtp,
            tc.tile_pool(name="mms", bufs=3) as mmsp,
            tc.tile_pool(name="prod", bufs=3) as prodp,
            tc.tile_pool(name="acc", bufs=2) as accp,
            tc.tile_pool(name="scr", bufs=1, space="PSUM") as scrp,
            tc.tile_pool(name="pst", bufs=3, space="PSUM") as pst,
            tc.tile_pool(name="psm", bufs=3, space="PSUM") as psm,
        ):
            nc.gpsimd.load_library(library_config.mlp)

            ident = constp.tile([128, 128], f32)
            nc.sync.dma_start(ident[:], id_d[:, :])
            w_sb = constp.tile([128, 128], f32)
            nc.sync.dma_start(w_sb[:], w_d[:, :])
            jx_sb = constp.tile([128, idx_cols], i16)
            nc.sync.dma_start(jx_sb[:], jx_d[:, :])
            ix_sb = constp.tile([128, idx_cols], i16)
            nc.sync.dma_start(ix_sb[:], ix_d[:, :])
            cnt_sb = constp.tile([1, nchunks], mybir.dt.int32)
            nc.sync.dma_start(cnt_sb[:], cnt_d[:, :])

            # dummy PE ops: absorb the constant-load DMA waits once
            scr = scrp.tile([128, 128], f32)
            nc.tensor.transpose(scr[:], ident[:], ident[:])
            scr2 = scrp.tile([128, 128], f32, tag="scr2")
            nc.tensor.matmul(scr2[:], lhsT=w_sb[:], rhs=w_sb[:],
                             start=True, stop=True)

            # valid-count register, reloaded per chunk (trailing -1 idx
            # padding is skipped by the gather ucode, saving descriptors)
            cnt_reg = nc.gpsimd.alloc_register("cnt_reg")

            base = 0
            qrr = [0]
            ci = [0]
            for cls in range(NCLS):
                bj, bi = divmod(cls, NSUB)
                zj_tab = z_d[bj * SUBROWS:(bj + 1) * SUBROWS, :]
                zi_tab = z_d[bi * SUBROWS:(bi + 1) * SUBROWS, :]
                for S in _chunks_of(caps[cls]):
                    slots = S // 128
                    cb = base // 16
                    nc.gpsimd.reg_load(cnt_reg, cnt_sb[:1, ci[0]:ci[0] + 1])
                    ci[0] += 1
                    gj = gp.tile([128, S], f32, tag="gj")
                    nc.gpsimd.dma_gather(
                        out_ap=gj[:].rearrange("p (s e) -> p s e", e=128),
                        in_ap=zj_tab,
                        idxs_ap=jx_sb[:, cb:cb + S // 16],
                        num_idxs=S,
                        num_idxs_reg=cnt_reg,
                        elem_size=128,
                        queue_num=0,
                    )
                    qrr[0] += 1
                    gi = gp.tile([128, S], f32, tag="gi")
                    nc.gpsimd.dma_gather(
                        out_ap=gi[:].rearrange("p (s e) -> p s e", e=128),
                        in_ap=zi_tab,
                        idxs_ap=ix_sb[:, cb:cb + S // 16],
                        num_idxs=S,
                        num_idxs_reg=cnt_reg,
                        elem_size=128,
                        queue_num=0,
                    )
                    qrr[0] += 1

                    logits = accp.tile([128, CHUNK // 128], f32, tag="logits")
                    for grp in range(slots // GRP):
                        tp = pst.tile([128, GRP * D], f32)
                        for u in range(GRP):
                            t = grp * GRP + u
                            nc.tensor.transpose(
                                tp[:, u * D:(u + 1) * D],
                                gj[:, t * D:(t + 1) * D],
                                ident[:],
                            )
                        tps = zjtp.tile([128, GRP * D], f32)
                        nc.scalar.copy(tps[:], tp[:])

                        mm = psm.tile([128, GRP * D], f32)
                        for u in range(GRP):
                            nc.tensor.matmul(
                                mm[:, u * D:(u + 1) * D],
                                lhsT=tps[:, u * D:(u + 1) * D],
                                rhs=w_sb[:],
                                start=True,
                                stop=True,
                            )
                        mms = mmsp.tile([128, GRP * D], f32)
                        nc.scalar.copy(mms[:], mm[:])

                        prod = prodp.tile([128, GRP * D], f32)
                        zi = gi[:, grp * GRP * D:(grp + 1) * GRP * D]
                        nc.vector.tensor_mul(out=prod[:], in0=zi, in1=mms[:])
                        nc.vector.reduce_sum(
                            out=logits[:, grp * GRP:(grp + 1) * GRP],
                            in_=prod[:].rearrange("p (u f) -> p u f", f=D),
                            axis=mybir.AxisListType.X,
                        )

                    probs = accp.tile([128, CHUNK // 128], f32, tag="probs")
                    nc.scalar.activation(
                        probs[:, :slots], logits[:, :slots],
                        mybir.ActivationFunctionType.Sigmoid,
                    )
                    nc.sync.dma_start(
                        out_d[:, base // 128:base // 128 + slots],
                        probs[:, :slots],
                    )
                    base += S

    return nc


def _get_program(caps, split):
    import concourse.mybir as mybir

    key = (tuple(caps), split)
    if key not in _CACHE:
        nc = _build_program(tuple(caps))
        _fix_gather_queues(nc)
        if split:
            _split_multi_waits(nc)
            # populate .instr bytes for InstISA subclasses (the library
            # reload); without this walrus fails with "ISA wrong length"
            mybir.codegen_inst_isa_subclasses(nc)
        _CACHE[key] = nc
    return _CACHE[key]


def _preprocess(z, edge_index, W):
    """Classify/pad edges per core; build per-core device inputs and the
    inverse mapping. Returns (caps, in_maps, perms)."""
    z = np.ascontiguousarray(np.asarray(z, dtype=np.float32))
    W = np.ascontiguousarray(np.asarray(W, dtype=np.float32))
    ident = np.eye(D, dtype=np.float32)
    ei = np.asarray(edge_index).astype(np.int64)
    jj_all = ei[1]
    ii_all = ei[0]

    per_core = []
    counts = np.zeros((NCORES, NCLS), np.int64)
    for c in range(NCORES):
        sl = slice(c * EPC, (c + 1) * EPC)
        jj = jj_all[sl]
        ii = ii_all[sl]
        cls = (jj // SUBROWS) * NSUB + (ii // SUBROWS)
        order = np.argsort(cls, kind="stable")
        counts[c] = np.bincount(cls, minlength=NCLS)
        per_core.append((jj, ii, cls, order))

    caps = counts.max(axis=0)
    caps = ((caps + PADQ - 1) // PADQ) * PADQ
    caps = tuple(int(x) for x in caps)
    tot = sum(caps)

    chunk_sizes = [S for k in range(NCLS) for S in _chunks_of(caps[k])]
    nchunks = len(chunk_sizes)

    in_maps = []
    perms = []
    for c in range(NCORES):
        jj, ii, cls, order = per_core[c]
        j16 = np.full(tot, -1, np.int16)
        i16 = np.full(tot, -1, np.int16)
        chunk_cnt = np.zeros(nchunks, np.int32)
        perm = np.full(tot, -1, np.int64)
        base = 0
        chunk_i = 0
        cnt = counts[c]
        cstart = np.zeros(NCLS + 1, np.int64)
        cstart[1:] = np.cumsum(cnt)
        for k in range(NCLS):
            ids = order[cstart[k]:cstart[k + 1]]
            n = len(ids)
            bj, bi = divmod(k, NSUB)
            j16[base:base + n] = (jj[ids] - bj * SUBROWS).astype(np.int16)
            i16[base:base + n] = (ii[ids] - bi * SUBROWS).astype(np.int16)
            perm[base:base + n] = ids
            # per-chunk valid counts (gather ucode skips trailing -1s);
            # every chunk needs >= 1 valid index
            off = 0
            for S in _chunks_of(caps[k]):
                v = min(max(n - off, 0), S)
                if v == 0:
                    j16[base + off] = 0
                    i16[base + off] = 0
                    v = 1
                chunk_cnt[chunk_i] = v
                chunk_i += 1
                off += S
            base += caps[k]
        # wrap into [16, tot/16] (position q -> [q%16, q//16]), replicate x8
        jw = np.tile(j16.reshape(-1, 16).T, (8, 1)).astype(np.int16)
        iw = np.tile(i16.reshape(-1, 16).T, (8, 1)).astype(np.int16)
        in_maps.append({
            "z": z, "w": W, "ident": ident,
            "jx": np.ascontiguousarray(jw),
            "ix": np.ascontiguousarray(iw),
            "cnt": chunk_cnt.reshape(1, -1),
        })
        perms.append(perm)
    return caps, in_maps, perms


def kernel(z, edge_index, W):
    from concourse.bass_utils import run_bass_kernel_spmd

    caps, in_maps, perms = _preprocess(z, edge_index, W)
    nc = _get_program(caps, split=True)
    res = run_bass_kernel_spmd(nc, in_maps, core_ids=list(range(NCORES)))
    out = np.empty(E, np.float32)
    for c in range(NCORES):
        o = res.results[c]["out"]          # [128, tot_slots]
        padded = o.T.ravel()               # padded position q = slot*128 + p
        perm = perms[c]
        valid = perm >= 0
        core_out = np.empty(EPC, np.float32)
        core_out[perm[valid]] = padded[valid]
        out[c * EPC:(c + 1) * EPC] = core_out
    return out
